# revision 1
# baseline (speedup 1.0000x reference)
"""Trainium2 Bass kernel for the SCAN-style cross-attention contrastive loss.

Sharding: image axis across 8 cores (8 images/core), captions replicated.
Each core computes its 66x8 column block of per-(caption,image) exp-sum
scores; the host gathers columns and applies the scalar hinge-loss epilogue.

Math restructure (validated to ~1e-7 against the jax reference):
  - unnormalized softmax weights u = exp(9*A_norm + wbias); the softmax
    denominator cancels in sim = num/(n1*||wctx||).
  - num  = E^T (u .* Araw)          (per-column reduction via indicator matmul)
  - q    = E^T (u .* (G_blk @ u)) = ||wctx_unnorm||^2 via per-caption Gram
  - invalid image frames are zeroed on host => their columns give e = 1
    exactly; host subtracts the known defect (F - img_len) from each exp-sum.
"""
from contextlib import ExitStack

import numpy as np

import concourse.bacc as bacc
import concourse.tile as tile
from concourse import mybir
from concourse.bass_utils import run_bass_kernel_spmd

N, F, W, D = 64, 64, 40, 512
NCORES = 8
IPC = N // NCORES        # images per core = 8
IF = IPC * F             # 512 image-frame columns per core
GP = 3                   # captions per partition group
NCAP = 66                # 64 captions padded to a multiple of GP
NG = NCAP // GP          # 22 groups
GW = GP * W              # 120 partitions per group
DCH = D // 128           # 4 contraction chunks
SG = 4                   # groups per normalization super-block

f32 = mybir.dt.float32
f32r = mybir.dt.float32r
FT = mybir.ActivationFunctionType
ALU = mybir.AluOpType
AX = mybir.AxisListType

MARGIN = 0.2
LAMBDA_LSE = 6.0


def _r(ap):
    return ap.bitcast(f32r)


def _build_nc():
    nc = bacc.Bacc("TRN2", target_bir_lowering=False, debug=False)
    capT = nc.dram_tensor("capT", [128, NG, DCH, GW], f32, kind="ExternalInput").ap()
    imgT = nc.dram_tensor("imgT", [128, DCH, IF], f32, kind="ExternalInput").ap()
    wbias = nc.dram_tensor("wbias", [GW, NG], f32, kind="ExternalInput").ap()
    gmask = nc.dram_tensor("gmask", [GW, GW], f32, kind="ExternalInput").ap()
    emat = nc.dram_tensor("emat", [GW, GP], f32, kind="ExternalInput").ap()
    ones = nc.dram_tensor("ones", [128, 1], f32, kind="ExternalInput").ap()
    se_out = nc.dram_tensor("se_out", [NCAP, IPC], f32, kind="ExternalOutput").ap()

    with tile.TileContext(nc) as tc, ExitStack() as ctx:
        const = ctx.enter_context(tc.tile_pool(name="const", bufs=1))
        caps = ctx.enter_context(tc.tile_pool(name="caps", bufs=3))
        work = ctx.enter_context(tc.tile_pool(name="work", bufs=2))
        small = ctx.enter_context(tc.tile_pool(name="small", bufs=3))
        pa = ctx.enter_context(tc.tile_pool(name="pa", bufs=2, space="PSUM"))
        pg = ctx.enter_context(tc.tile_pool(name="pg", bufs=1, space="PSUM"))
        pb = ctx.enter_context(tc.tile_pool(name="pb", bufs=1, space="PSUM"))
        pq = ctx.enter_context(tc.tile_pool(name="pq", bufs=2, space="PSUM"))

        imgT_t = const.tile([128, DCH, IF], f32r)
        nc.sync.dma_start(out=imgT_t, in_=imgT.bitcast(f32r))
        wbias_t = const.tile([GW, NG], f32)
        nc.sync.dma_start(out=wbias_t, in_=wbias)
        gmask_t = const.tile([GW, GW], f32)
        nc.sync.dma_start(out=gmask_t, in_=gmask)
        emat_t = const.tile([GW, GP], f32r)
        nc.sync.dma_start(out=emat_t, in_=emat.bitcast(f32r))
        ones_col = const.tile([128, 1], f32r)
        nc.sync.dma_start(out=ones_col, in_=ones.bitcast(f32r))
        eps_col = const.tile([128, 1], f32)
        nc.vector.memset(eps_col, 1e-20)

        # n1sq[f] = ||image frame f||^2, replicated across partitions
        imgsq_t = const.tile([128, DCH, IF], f32r)
        nc.vector.tensor_mul(imgsq_t, imgT_t.bitcast(f32), imgT_t.bitcast(f32))
        n1p = pg.tile([1, IF], f32, tag="g")
        for c in range(DCH):
            nc.tensor.matmul(out=n1p, lhsT=ones_col, rhs=imgsq_t[:, c, :],
                             start=(c == 0), stop=(c == DCH - 1))
        n1row = const.tile([1, IF], f32)
        nc.scalar.copy(n1row, n1p)
        n1repl = const.tile([128, IF], f32)
        nc.gpsimd.partition_broadcast(n1repl, n1row[0:1, :])

        # q|num gathered into one SBUF accumulator via ACT staging + DMAs
        qn_all = const.tile([NCAP, 2, IF], f32)

        SBS = [(s, min(SG, NG - s)) for s in range(0, NG, SG)]
        for s0, sbn in SBS:
            r2sb = small.tile([GW, SG, IPC], f32, tag="r2sb")
            srsb = small.tile([GW, SG, IPC], f32, tag="srsb")
            rinvsb = small.tile([GW, SG, IPC], f32, tag="rinvsb")
            held = []
            for j in range(sbn):
                g = s0 + j
                capg = caps.tile([128, DCH, GW], f32r, bufs=4)
                nc.sync.dma_start(out=capg, in_=capT[:, g, :, :].bitcast(f32r))

                # Araw[w, if] = caption_word . image_frame
                araw_p = pa.tile([GW, IF], f32)
                for c in range(DCH):
                    nc.tensor.matmul(out=araw_p, lhsT=capg[:, c, :],
                                     rhs=imgT_t[:, c, :],
                                     start=(c == 0), stop=(c == DCH - 1))

                # per-caption Gram (block-diagonal after masking)
                gram_p = pg.tile([GW, GW], f32, tag="g")
                for c in range(DCH):
                    nc.tensor.matmul(out=gram_p, lhsT=capg[:, c, :],
                                     rhs=capg[:, c, :],
                                     start=(c == 0), stop=(c == DCH - 1))
                gblk_t = work.tile([GW, GW], f32r, bufs=SG + 1)
                nc.vector.tensor_mul(gblk_t, gram_p, gmask_t)

                araw_t = work.tile([GW, IF], f32, bufs=SG + 1)
                nc.scalar.copy(araw_t, araw_p)

                # leaky relu: L = max(0.1*A, A)
                L_t = work.tile([GW, IF], f32, bufs=SG + 1)
                nc.vector.scalar_tensor_tensor(out=L_t, in0=araw_t, scalar=0.1,
                                               in1=araw_t, op0=ALU.mult,
                                               op1=ALU.max)

                sq_t = work.tile([GW, IF], f32, bufs=3)
                nc.gpsimd.tensor_mul(sq_t, L_t, L_t)
                nc.vector.reduce_sum(r2sb[:, j, :],
                                     sq_t.rearrange("p (i f) -> p i f", f=F),
                                     axis=AX.X)
                held.append((araw_t, L_t, gblk_t))

            # one sqrt + one reciprocal per super-block
            nc.scalar.activation(srsb[:, 0:sbn, :], r2sb[:, 0:sbn, :], FT.Sqrt)
            nc.vector.reciprocal(rinvsb[:, 0:sbn, :], srsb[:, 0:sbn, :])

            for j in range(sbn):
                g = s0 + j
                araw_t, L_t, gblk_t = held[j]
                at_t = work.tile([GW, IF], f32, bufs=3)
                nc.vector.tensor_mul(at_t.rearrange("p (i f) -> p i f", f=F),
                                     L_t.rearrange("p (i f) -> p i f", f=F),
                                     rinvsb[:, j, :].to_broadcast([GW, IPC, F]))
                u_t = work.tile([GW, IF], f32r, bufs=3)
                nc.scalar.activation(u_t, at_t, FT.Exp, scale=9.0,
                                     bias=wbias_t[:, g:g + 1])

                b_p = pb.tile([GW, IF], f32)
                nc.tensor.matmul(out=b_p, lhsT=gblk_t, rhs=u_t,
                                 start=True, stop=True)

                p_t = work.tile([GW, IF], f32r, bufs=3)
                nc.vector.tensor_mul(p_t, u_t.bitcast(f32), b_p)
                q_t = work.tile([GW, IF], f32r, bufs=3)
                nc.gpsimd.tensor_mul(q_t, u_t.bitcast(f32), araw_t)

                qnp = pq.tile([GP, 2, IF], f32, tag="qn")
                nc.tensor.matmul(out=qnp[:, 0, :], lhsT=emat_t, rhs=p_t,
                                 start=True, stop=True)
                nc.tensor.matmul(out=qnp[:, 1, :], lhsT=emat_t, rhs=q_t,
                                 start=True, stop=True)
                qnstg = small.tile([GP, 2, IF], f32, tag="qnstg")
                nc.scalar.copy(qnstg, qnp)
                nc.sync.dma_start(out=qn_all[g * GP:(g + 1) * GP, :, :],
                                  in_=qnstg)

        # single kernel-wide epilogue:
        # sim = num / sqrt(q * n1sq), e = exp(6*sim), block-sum over frames
        qs_t = work.tile([NCAP, IF], f32, tag="qs_t")
        nc.vector.tensor_mul(qs_t, qn_all[:, 0, :], n1repl[0:NCAP, :])
        d_t = work.tile([NCAP, IF], f32, tag="d_t")
        nc.scalar.activation(d_t, qs_t, FT.Sqrt, bias=eps_col[0:NCAP, :])
        ri2_t = work.tile([NCAP, IF], f32, tag="ri2_t")
        nc.vector.reciprocal(ri2_t, d_t)
        sim_t = work.tile([NCAP, IF], f32, tag="sim_t")
        nc.vector.tensor_mul(sim_t, qn_all[:, 1, :], ri2_t)
        e_t = work.tile([NCAP, IF], f32, tag="e_t")
        nc.scalar.activation(e_t, sim_t, FT.Exp, scale=LAMBDA_LSE)
        seg_t = small.tile([NCAP, IPC], f32, tag="seg_t")
        nc.vector.reduce_sum(seg_t, e_t.rearrange("p (i f) -> p i f", f=F),
                             axis=AX.X)
        nc.sync.dma_start(out=se_out, in_=seg_t)

    nc.compile()
    return nc


_NC = None


def _get_nc():
    global _NC
    if _NC is None:
        _NC = _build_nc()
    return _NC


def make_in_maps(images, captions, img_lens, cap_lens):
    """Host-side input preparation (numpy only): shard/transpose/mask."""
    images = np.ascontiguousarray(np.asarray(images, np.float32))
    captions = np.ascontiguousarray(np.asarray(captions, np.float32))
    img_lens = np.asarray(img_lens).astype(np.int64)
    cap_lens = np.asarray(cap_lens).astype(np.int64)

    # captions padded to 66; dummies replicate caption 0 (avoids 0/0)
    caps_p = np.concatenate(
        [captions, np.broadcast_to(captions[0:1], (NCAP - N, W, D))], axis=0)
    # [128, NG, DCH, GW] with partition = d % 128, GW index = b*W + w
    capT_np = np.ascontiguousarray(
        caps_p.reshape(NG, GP, W, DCH, 128).transpose(4, 0, 3, 1, 2)
        .reshape(128, NG, DCH, GW))

    wbias_np = np.full((NCAP, W), np.float32(-1e30))
    for j in range(N):
        wbias_np[j, :cap_lens[j]] = 0.0
    wbias_np = np.ascontiguousarray(
        wbias_np.reshape(NG, GP * W).T.astype(np.float32))  # [GW, NG]

    gmask_np = np.zeros((GW, GW), np.float32)
    emat_np = np.zeros((GW, GP), np.float32)
    for b in range(GP):
        gmask_np[b * W:(b + 1) * W, b * W:(b + 1) * W] = 1.0
        emat_np[b * W:(b + 1) * W, b] = 1.0

    in_maps = []
    for core in range(NCORES):
        imgs = images[core * IPC:(core + 1) * IPC].copy()
        for i in range(IPC):
            imgs[i, img_lens[core * IPC + i]:] = 0.0
        Z = imgs.reshape(IF, D)
        imgT_np = np.ascontiguousarray(
            Z.reshape(IF, DCH, 128).transpose(2, 1, 0))  # [128, DCH, IF]
        in_maps.append({
            "capT": capT_np, "imgT": imgT_np, "wbias": wbias_np,
            "gmask": gmask_np, "emat": emat_np,
            "ones": np.ones((128, 1), np.float32),
        })
    return in_maps


def finish(se_list, img_lens):
    """Host epilogue: defect correction, log-sum-exp, hinge loss."""
    img_lens = np.asarray(img_lens).astype(np.int64)
    cols = []
    for core in range(NCORES):
        se = np.asarray(se_list[core], np.float32)[:N, :]         # (64, 8)
        defect = (F - img_lens[core * IPC:(core + 1) * IPC]).astype(np.float32)
        cols.append(np.log(se - defect[None, :]) / LAMBDA_LSE)
    S = np.concatenate(cols, axis=1).astype(np.float32)           # (caps, imgs)

    diag = np.diag(S)
    eye = np.eye(N, dtype=bool)
    cost_s = np.maximum(MARGIN + S - diag[:, None], 0.0)
    cost_im = np.maximum(MARGIN + S - diag[None, :], 0.0)
    cost_s[eye] = 0.0
    cost_im[eye] = 0.0
    return np.float32(cost_s.max(axis=1).sum() + cost_im.max(axis=0).sum())


def kernel(images, captions, img_lens, cap_lens):
    nc = _get_nc()
    in_maps = make_in_maps(images, captions, img_lens, cap_lens)
    res = run_bass_kernel_spmd(nc, in_maps, core_ids=list(range(NCORES)))
    se_list = [res.results[c]["se_out"] for c in range(NCORES)]
    return finish(se_list, img_lens)



# revision 5
# speedup vs baseline: 1.0503x; 1.0503x over previous
"""Trainium2 Bass kernel for the SCAN-style cross-attention contrastive loss.

Sharding: image axis across 8 cores (8 images/core), captions replicated.
Each core computes its 66x8 column block of per-(caption,image) exp-sum
scores; the host gathers columns and applies the scalar hinge-loss epilogue.

Math restructure (same as validated baseline, now in bf16):
  - unnormalized softmax weights u = exp(9*A_norm + wbias); the softmax
    denominator cancels in sim = num/(n1*||wctx||).
  - num  = E^T (u .* Araw)          (per-column reduction via indicator matmul)
  - q    = E^T (u .* (G_blk @ u)) = ||wctx_unnorm||^2 via per-caption Gram
  - invalid image frames are zeroed on host => their columns give e = 1
    exactly; host subtracts the known defect (F - img_len) from each exp-sum.

v2 performance structure:
  - all matmul operands bf16 (hosts converts); Gram blocks precomputed on host
  - one ACT table set (natural_log_exp_and_others): Prelu for leaky-relu,
    Exp, and rsqrt as exp(-0.5*ln(x)) -- zero table switches
  - per-group (caption-pair, image-frame) E-matmuls accumulate num/q into one
    persistent PSUM region via per-group wide indicator weights (no staging)
  - software-pipelined pair loop; engines balanced:
      ACT: lrelu, a-copy, rinv(ln+exp), exp(u)
      DVE: sq, frame-reduce, at=L*rinv, p=u*b
      GPS: rinv broadcast-expand, q=u*a
      PE : Araw (4 chunks), b=G@u, 2x E-matmul
"""
from contextlib import ExitStack

import numpy as np
import ml_dtypes

import concourse.bacc as bacc
import concourse.tile as tile
from concourse import mybir
from concourse.bass_utils import run_bass_kernel_spmd

N, F, W, D = 64, 64, 40, 512
NCORES = 8
IPC = N // NCORES        # images per core = 8
IF = IPC * F             # 512 image-frame columns per core
GP = 3                   # captions per partition group
NCAP = 66                # 64 captions padded to a multiple of GP
NG = NCAP // GP          # 22 groups
GW = GP * W              # 120 partitions per group
DCH = D // 128           # 4 contraction chunks
PKW = DCH * GW + GW      # packed group width: 480 capT cols + 120 gram cols

f32 = mybir.dt.float32
bf16 = mybir.dt.bfloat16
FT = mybir.ActivationFunctionType
ALU = mybir.AluOpType
AX = mybir.AxisListType
BF16NP = ml_dtypes.bfloat16

MARGIN = 0.2
LAMBDA_LSE = 6.0


def _build_nc():
    nc = bacc.Bacc("TRN2", target_bir_lowering=False, debug=False)
    imgT = nc.dram_tensor("imgT", [128, DCH, IF], bf16, kind="ExternalInput").ap()
    packed = nc.dram_tensor("packed", [128, NG, PKW], bf16, kind="ExternalInput").ap()
    wbias = nc.dram_tensor("wbias", [GW, NG], f32, kind="ExternalInput").ap()
    eall = nc.dram_tensor("eall", [GW, NG, NCAP], bf16, kind="ExternalInput").ap()
    n1sq = nc.dram_tensor("n1sq", [NCAP, IF], f32, kind="ExternalInput").ap()
    se_out = nc.dram_tensor("se_out", [NCAP, IPC], f32, kind="ExternalOutput").ap()

    NPAIR = (NG + 1) // 2

    with tile.TileContext(nc) as tc, ExitStack() as ctx:
        const = ctx.enter_context(tc.tile_pool(name="const", bufs=1))
        pkp = ctx.enter_context(tc.tile_pool(name="pkp", bufs=6))
        lp = ctx.enter_context(tc.tile_pool(name="lp", bufs=4))
        acp = ctx.enter_context(tc.tile_pool(name="acp", bufs=4))
        sqp = ctx.enter_context(tc.tile_pool(name="sqp", bufs=3))
        smal = ctx.enter_context(tc.tile_pool(name="smal", bufs=3))
        rvxp = ctx.enter_context(tc.tile_pool(name="rvxp", bufs=3))
        up = ctx.enter_context(tc.tile_pool(name="up", bufs=3))
        pqp = ctx.enter_context(tc.tile_pool(name="pqp", bufs=3))
        epi = ctx.enter_context(tc.tile_pool(name="epi", bufs=1))
        pa = ctx.enter_context(tc.tile_pool(name="pa", bufs=4, space="PSUM"))
        pb = ctx.enter_context(tc.tile_pool(name="pb", bufs=2, space="PSUM"))
        pqn = ctx.enter_context(tc.tile_pool(name="pqn", bufs=1, space="PSUM"))

        imgT_t = const.tile([128, DCH, IF], bf16)
        nc.sync.dma_start(out=imgT_t, in_=imgT)
        wbias_t = const.tile([GW, NG], f32)
        nc.sync.dma_start(out=wbias_t, in_=wbias)
        eall_t = const.tile([GW, NG, NCAP], bf16)
        nc.sync.dma_start(out=eall_t, in_=eall)
        n1sq_t = const.tile([NCAP, IF], f32)
        nc.sync.dma_start(out=n1sq_t, in_=n1sq)
        eps_col = const.tile([128, 1], f32)
        nc.vector.memset(eps_col, 1e-30)

        # persistent PSUM accumulator: [:, 0, :] = q (u^T G u), [:, 1, :] = num
        qn_ps = pqn.tile([NCAP, 2, IF], f32)

        pk_t = [None] * NG
        L_t = [None] * NG
        ac_t = [None] * NG
        araw_p = [None] * NG
        r2_t = [None] * NPAIR
        rv_t = [None] * NPAIR

        def dma_group(g):
            pk_t[g] = pkp.tile([128, PKW], bf16, tag="pk", name=f"pk{g}")
            nc.sync.dma_start(out=pk_t[g], in_=packed[:, g, :])

        def stage1(g):
            """Araw matmuls + lrelu + raw copy + square + frame-reduce."""
            k, j = g // 2, g % 2
            araw_p[g] = pa.tile([GW, IF], f32, tag="araw", name=f"araw{g}")
            for c in range(DCH):
                nc.tensor.matmul(out=araw_p[g],
                                 lhsT=pk_t[g][:, c * GW:(c + 1) * GW],
                                 rhs=imgT_t[:, c, :],
                                 start=(c == 0), stop=(c == DCH - 1))
            L_t[g] = lp.tile([GW, IPC, F], bf16, tag="L", name=f"L{g}")
            nc.scalar.activation(L_t[g].rearrange("p i f -> p (i f)"), araw_p[g],
                                 FT.Prelu, alpha=0.1)
            ac_t[g] = acp.tile([GW, IF], bf16, tag="ac", name=f"ac{g}")
            nc.scalar.activation(ac_t[g], araw_p[g], FT.Copy)
            if j == 0:
                r2_t[k] = smal.tile([GW, 2, IPC], f32, tag="r2", name=f"r2_{k}")
            sq_t = sqp.tile([GW, IPC, F], bf16, tag="sq")
            nc.vector.tensor_mul(sq_t, L_t[g], L_t[g])
            nc.vector.reduce_sum(r2_t[k][:, j, :], sq_t, axis=AX.X)

        def rinv(k):
            """rinv = r2^-0.5 for a pair of groups via ln+exp (same ACT set)."""
            ln_t = smal.tile([GW, 2, IPC], f32, tag="ln", name=f"ln{k}")
            nc.scalar.activation(ln_t, r2_t[k], FT.Ln, bias=eps_col[0:GW, :])
            rv_t[k] = smal.tile([GW, 2, IPC], bf16, tag="rv", name=f"rv{k}")
            nc.scalar.activation(rv_t[k], ln_t, FT.Exp, scale=-0.5)

        def stage2(g):
            """at = L*rinv, u = exp(9at+bias), b = G@u, p = u*b, q = u*a,
            and the accumulating per-caption E-matmuls."""
            k, j = g // 2, g % 2
            rvx_t = rvxp.tile([GW, IPC, F], bf16, tag="rvx")
            nc.gpsimd.tensor_copy(rvx_t, rv_t[k][:, j, :].to_broadcast([GW, IPC, F]))
            at_t = rvxp.tile([GW, IPC, F], bf16, tag="at")
            nc.vector.tensor_mul(at_t, L_t[g], rvx_t)
            u_t = up.tile([GW, IF], bf16, tag="u")
            nc.scalar.activation(u_t, at_t.rearrange("p i f -> p (i f)"), FT.Exp,
                                 scale=9.0, bias=wbias_t[:, g:g + 1])
            b_p = pb.tile([GW, IF], f32, tag="b")
            nc.tensor.matmul(out=b_p, lhsT=pk_t[g][0:GW, DCH * GW:], rhs=u_t,
                             start=True, stop=True)
            pq_t = pqp.tile([GW, 2, IF], bf16, tag="pq")
            nc.vector.tensor_mul(pq_t[:, 0, :], u_t, b_p)
            nc.gpsimd.tensor_mul(pq_t[:, 1, :], u_t, ac_t[g])
            for s in range(2):
                nc.tensor.matmul(out=qn_ps[:, s, :], lhsT=eall_t[:, g, :],
                                 rhs=pq_t[:, s, :],
                                 start=(g == 0), stop=(g == NG - 1))

        # software-pipelined main loop (pairs of caption groups)
        for g in range(min(6, NG)):
            dma_group(g)
        stage1(0)
        if NG > 1:
            stage1(1)
        rinv(0)
        for k in range(NPAIR):
            for g in (2 * k + 6, 2 * k + 7):
                if g < NG:
                    dma_group(g)
            for g in (2 * k + 2, 2 * k + 3):
                if g < NG:
                    stage1(g)
            if k + 1 < NPAIR:
                rinv(k + 1)
            for g in (2 * k, 2 * k + 1):
                if g < NG:
                    stage2(g)

        # epilogue: sim = num * (q*n1sq)^-0.5, e = exp(6 sim), sum over frames
        qs_t = epi.tile([NCAP, IF], f32)
        nc.vector.tensor_mul(qs_t, qn_ps[:, 0, :], n1sq_t)
        lq_t = epi.tile([NCAP, IF], f32)
        nc.scalar.activation(lq_t, qs_t, FT.Ln, bias=eps_col[0:NCAP, :])
        rq_t = epi.tile([NCAP, IF], f32)
        nc.scalar.activation(rq_t, lq_t, FT.Exp, scale=-0.5)
        sim_t = epi.tile([NCAP, IPC, F], f32)
        nc.vector.tensor_mul(sim_t.rearrange("p i f -> p (i f)"), qn_ps[:, 1, :],
                             rq_t)
        e_t = epi.tile([NCAP, IPC, F], f32)
        nc.scalar.activation(e_t, sim_t, FT.Exp, scale=LAMBDA_LSE)
        se_t = epi.tile([NCAP, IPC], f32)
        nc.vector.reduce_sum(se_t, e_t, axis=AX.X)
        nc.sync.dma_start(out=se_out, in_=se_t)

    nc.compile()
    return nc


_NC = None


def _get_nc():
    global _NC
    if _NC is None:
        _NC = _build_nc()
    return _NC


def make_in_maps(images, captions, img_lens, cap_lens):
    """Host-side input preparation (numpy only): shard/transpose/mask."""
    images = np.ascontiguousarray(np.asarray(images, np.float32))
    captions = np.ascontiguousarray(np.asarray(captions, np.float32))
    img_lens = np.asarray(img_lens).astype(np.int64)
    cap_lens = np.asarray(cap_lens).astype(np.int64)

    # captions padded to 66; dummies replicate caption 0 (avoids 0/0)
    caps_p = np.concatenate(
        [captions, np.broadcast_to(captions[0:1], (NCAP - N, W, D))], axis=0)
    caps_bf = caps_p.astype(BF16NP)
    # capT view [128, NG, DCH, GP*W]: partition = d % 128 within chunk
    capT_np = np.ascontiguousarray(
        caps_bf.reshape(NG, GP, W, DCH, 128).transpose(4, 0, 3, 1, 2)
        .reshape(128, NG, DCH * GW))

    # per-caption Gram blocks from the bf16-rounded captions (consistency
    # with the on-device bf16 matmuls); block-diagonal per 3-caption group
    cf = caps_bf.astype(np.float32)
    G = np.einsum('jwd,jvd->jwv', cf, cf)                  # (66, 40, 40)
    gpk = np.zeros((128, NG, GW), np.float32)
    for g in range(NG):
        for b in range(GP):
            gpk[b * W:(b + 1) * W, g, b * W:(b + 1) * W] = G[g * GP + b]
    packed_np = np.concatenate([capT_np, gpk.astype(BF16NP)], axis=2)

    wbias_np = np.full((NCAP, W), np.float32(-1e30))
    for j in range(N):
        wbias_np[j, :cap_lens[j]] = 0.0
    wbias_np = np.ascontiguousarray(
        wbias_np.reshape(NG, GP * W).T.astype(np.float32))  # [GW, NG]

    # per-group wide indicator: group g's caption b sums into output row 3g+b
    eall_np = np.zeros((GW, NG, NCAP), np.float32)
    for g in range(NG):
        for b in range(GP):
            eall_np[b * W:(b + 1) * W, g, g * GP + b] = 1.0
    eall_np = eall_np.astype(BF16NP)

    in_maps = []
    for core in range(NCORES):
        imgs = images[core * IPC:(core + 1) * IPC].copy()
        for i in range(IPC):
            imgs[i, img_lens[core * IPC + i]:] = 0.0
        imgs_bf = imgs.astype(BF16NP)
        Z = imgs_bf.reshape(IF, D)
        imgT_np = np.ascontiguousarray(
            Z.reshape(IF, DCH, 128).transpose(2, 1, 0))  # [128, DCH, IF]
        n1 = (Z.astype(np.float32) ** 2).sum(axis=1)     # [IF]
        n1sq_np = np.ascontiguousarray(
            np.broadcast_to(n1[None, :], (NCAP, IF)).astype(np.float32))
        in_maps.append({
            "imgT": imgT_np, "packed": packed_np, "wbias": wbias_np,
            "eall": eall_np, "n1sq": n1sq_np,
        })
    return in_maps


def finish(se_list, img_lens):
    """Host epilogue: defect correction, log-sum-exp, hinge loss."""
    img_lens = np.asarray(img_lens).astype(np.int64)
    cols = []
    for core in range(NCORES):
        se = np.asarray(se_list[core], np.float32)[:N, :]         # (64, 8)
        defect = (F - img_lens[core * IPC:(core + 1) * IPC]).astype(np.float32)
        cols.append(np.log(se - defect[None, :]) / LAMBDA_LSE)
    S = np.concatenate(cols, axis=1).astype(np.float32)           # (caps, imgs)

    diag = np.diag(S)
    eye = np.eye(N, dtype=bool)
    cost_s = np.maximum(MARGIN + S - diag[:, None], 0.0)
    cost_im = np.maximum(MARGIN + S - diag[None, :], 0.0)
    cost_s[eye] = 0.0
    cost_im[eye] = 0.0
    return np.float32(cost_s.max(axis=1).sum() + cost_im.max(axis=0).sum())


def kernel(images, captions, img_lens, cap_lens):
    nc = _get_nc()
    in_maps = make_in_maps(images, captions, img_lens, cap_lens)
    res = run_bass_kernel_spmd(nc, in_maps, core_ids=list(range(NCORES)))
    se_list = [res.results[c]["se_out"] for c in range(NCORES)]
    return finish(se_list, img_lens)


# revision 6
# speedup vs baseline: 1.5299x; 1.4566x over previous
"""Trainium2 Bass kernel for the SCAN-style cross-attention contrastive loss.

Sharding: image axis across 8 cores (8 images/core), captions replicated.
Each core computes its 66x8 column block of per-(caption,image) exp-sum
scores; the host gathers columns and applies the scalar hinge-loss epilogue.

Math restructure (validated against the jax reference):
  - unnormalized softmax weights u = exp(9*A_norm + wbias); the softmax
    denominator cancels in sim = num/(n1*||wctx||).
  - num  = E^T (u .* Araw)          (per-column reduction via indicator matmul)
  - q    = E^T (u .* (G_blk @ u)) = ||wctx_unnorm||^2 via per-caption Gram
  - invalid image frames are zeroed on host => their columns give e = 1
    exactly; host subtracts the known defect (F - img_len) from each exp-sum.

Performance structure:
  - all matmul operands bf16; weights padded to 128 columns (enables FWL);
    per-caption Gram blocks precomputed on host
  - ONE ACT table set (natural_log_exp_and_others, forced via the table map
    the load-insertion pass consults): Prelu = leaky-relu, Exp, and
    rsqrt(x) = exp(-0.5*ln(x)) -- zero mid-kernel table switches
  - image-frame columns are f-major (col = f*IPC + i) so the per-(word,image)
    rinv broadcast has a step-1 innermost axis -> bf16 2x DVE mode
  - per-group E-matmuls accumulate num/q into one persistent PSUM region
  - software-pipelined pair loop; engines balanced:
      ACT: lrelu, a-copy, rinv(ln+exp), exp(u)
      DVE: sq, frame-reduce, at=L*rinv, p=u*b
      GPS: q=u*a
      PE : Araw (4 chunks), b=G@u, 2x E-matmul
"""
from contextlib import ExitStack

import numpy as np
import ml_dtypes

import concourse.bacc as bacc
from concourse import hw_specs as _hw_specs
import concourse.tile as tile
from concourse import mybir
from concourse.bass_utils import run_bass_kernel_spmd

# Force every ACT instruction to resolve to the one table set that contains
# all functions we use (parametric_relu, copy, exp, ln). Set indexes are
# preserved, so the runtime id mapping stays valid; this only stops the
# load-insertion pass from ping-ponging between exp/ln anchor sets.
_JOINT_ACT_SET = "natural_log_exp_and_others"
_orig_get_tables = _hw_specs.get_activation_tables


def _forced_tables(arch):
    tabs = _orig_get_tables(arch)
    assert _JOINT_ACT_SET in tabs
    return {k: (v if k == _JOINT_ACT_SET else set()) for k, v in tabs.items()}


bacc.get_activation_tables = _forced_tables

N, F, W, D = 64, 64, 40, 512
NCORES = 8
IPC = N // NCORES        # images per core = 8
IF = IPC * F             # 512 image-frame columns per core (f-major order)
GP = 3                   # captions per partition group
NCAP = 66                # 64 captions padded to a multiple of GP
NG = NCAP // GP          # 22 groups
GW = GP * W              # 120 real partitions per group (padded to 128)
DCH = D // 128           # 4 contraction chunks
PKW = DCH * 128 + 128    # packed group width: 4x128 capT cols + 128 gram cols

f32 = mybir.dt.float32
bf16 = mybir.dt.bfloat16
FT = mybir.ActivationFunctionType
ALU = mybir.AluOpType
AX = mybir.AxisListType
BF16NP = ml_dtypes.bfloat16

MARGIN = 0.2
LAMBDA_LSE = 6.0


def _build_nc():
    nc = bacc.Bacc("TRN2", target_bir_lowering=False, debug=False)
    imgT = nc.dram_tensor("imgT", [128, DCH, IF], bf16, kind="ExternalInput").ap()
    packed = nc.dram_tensor("packed", [128, NG, PKW], bf16, kind="ExternalInput").ap()
    wbias = nc.dram_tensor("wbias", [128, NG], f32, kind="ExternalInput").ap()
    eall = nc.dram_tensor("eall", [128, NG, 128], bf16, kind="ExternalInput").ap()
    n1sq = nc.dram_tensor("n1sq", [NCAP, IF], f32, kind="ExternalInput").ap()
    se_out = nc.dram_tensor("se_out", [NCAP, IPC], f32, kind="ExternalOutput").ap()

    NPAIR = (NG + 1) // 2

    with tile.TileContext(nc) as tc, ExitStack() as ctx:
        const = ctx.enter_context(tc.tile_pool(name="const", bufs=1))
        pkp = ctx.enter_context(tc.tile_pool(name="pkp", bufs=6))
        lp = ctx.enter_context(tc.tile_pool(name="lp", bufs=4))
        acp = ctx.enter_context(tc.tile_pool(name="acp", bufs=4))
        sqp = ctx.enter_context(tc.tile_pool(name="sqp", bufs=3))
        smal = ctx.enter_context(tc.tile_pool(name="smal", bufs=3))
        up = ctx.enter_context(tc.tile_pool(name="up", bufs=3))
        pqp = ctx.enter_context(tc.tile_pool(name="pqp", bufs=3))
        epi = ctx.enter_context(tc.tile_pool(name="epi", bufs=1))
        pa = ctx.enter_context(tc.tile_pool(name="pa", bufs=4, space="PSUM"))
        pb = ctx.enter_context(tc.tile_pool(name="pb", bufs=2, space="PSUM"))
        pqn = ctx.enter_context(tc.tile_pool(name="pqn", bufs=1, space="PSUM"))

        imgT_t = const.tile([128, DCH, IF], bf16)
        nc.sync.dma_start(out=imgT_t, in_=imgT)
        wbias_t = const.tile([128, NG], f32)
        nc.sync.dma_start(out=wbias_t, in_=wbias)
        eall_t = const.tile([128, NG, 128], bf16)
        nc.sync.dma_start(out=eall_t, in_=eall)
        n1sq_t = const.tile([NCAP, IF], f32)
        nc.sync.dma_start(out=n1sq_t, in_=n1sq)
        eps_col = const.tile([128, 1], f32)
        nc.vector.memset(eps_col, 1e-30)

        # persistent PSUM accumulator: [:, 0, :] = q (u^T G u), [:, 1, :] = num
        qn_ps = pqn.tile([128, 2, IF], f32)

        pk_t = [None] * NG
        L_t = [None] * NG
        ac_t = [None] * NG
        araw_p = [None] * NG
        r2_t = [None] * NPAIR
        rv_t = [None] * NPAIR

        def dma_group(g):
            pk_t[g] = pkp.tile([128, PKW], bf16, tag="pk", name=f"pk{g}")
            nc.sync.dma_start(out=pk_t[g], in_=packed[:, g, :])

        def stage1(g):
            """Araw matmuls + lrelu + raw copy + square + frame-reduce."""
            k, j = g // 2, g % 2
            araw_p[g] = pa.tile([128, IF], f32, tag="araw", name=f"araw{g}")
            for c in range(DCH):
                nc.tensor.matmul(out=araw_p[g],
                                 lhsT=pk_t[g][:, c * 128:(c + 1) * 128],
                                 rhs=imgT_t[:, c, :],
                                 start=(c == 0), stop=(c == DCH - 1))
            L_t[g] = lp.tile([128, IF], bf16, tag="L", name=f"L{g}")
            nc.scalar.activation(L_t[g], araw_p[g], FT.Prelu, alpha=0.1)
            ac_t[g] = acp.tile([128, IF], bf16, tag="ac", name=f"ac{g}")
            nc.scalar.activation(ac_t[g], araw_p[g], FT.Copy)
            if j == 0:
                r2_t[k] = smal.tile([128, 2, IPC], f32, tag="r2", name=f"r2_{k}")
            sq_t = sqp.tile([128, IF], bf16, tag="sq")
            nc.vector.tensor_mul(sq_t, L_t[g], L_t[g])
            nc.vector.reduce_sum(r2_t[k][:, j, :],
                                 sq_t.rearrange("p (f i) -> p i f", i=IPC),
                                 axis=AX.X)

        def rinv(k):
            """rinv = r2^-0.5 for a pair of groups via ln+exp (same ACT set)."""
            ln_t = smal.tile([128, 2, IPC], f32, tag="ln", name=f"ln{k}")
            nc.scalar.activation(ln_t, r2_t[k], FT.Ln, bias=eps_col)
            rv_t[k] = smal.tile([128, 2, IPC], bf16, tag="rv", name=f"rv{k}")
            nc.scalar.activation(rv_t[k], ln_t, FT.Exp, scale=-0.5)

        def stage2(g):
            """at = L*rinv, u = exp(9at+bias), b = G@u, p = u*b, q = u*a,
            and the accumulating per-caption E-matmuls."""
            k, j = g // 2, g % 2
            at_t = up.tile([128, F, IPC], bf16, tag="at")
            rvb = rv_t[k][:, j, :].unsqueeze(1).broadcast_to([128, F, IPC])
            nc.vector.tensor_mul(at_t, L_t[g].rearrange("p (f i) -> p f i", i=IPC),
                                 rvb)
            u_t = up.tile([128, IF], bf16, tag="u")
            nc.scalar.activation(u_t, at_t.rearrange("p f i -> p (f i)"), FT.Exp,
                                 scale=9.0, bias=wbias_t[:, g:g + 1])
            b_p = pb.tile([128, IF], f32, tag="b")
            nc.tensor.matmul(out=b_p, lhsT=pk_t[g][:, DCH * 128:], rhs=u_t,
                             start=True, stop=True)
            pq_t = pqp.tile([128, 2, IF], bf16, tag="pq")
            nc.vector.tensor_mul(pq_t[:, 0, :], u_t, b_p)
            nc.gpsimd.tensor_mul(pq_t[:, 1, :], u_t, ac_t[g])
            for s in range(2):
                nc.tensor.matmul(out=qn_ps[:, s, :], lhsT=eall_t[:, g, :],
                                 rhs=pq_t[:, s, :],
                                 start=(g == 0), stop=(g == NG - 1))

        # software-pipelined main loop (pairs of caption groups)
        for g in range(min(6, NG)):
            dma_group(g)
        stage1(0)
        if NG > 1:
            stage1(1)
        rinv(0)
        for k in range(NPAIR):
            for g in (2 * k + 6, 2 * k + 7):
                if g < NG:
                    dma_group(g)
            for g in (2 * k + 2, 2 * k + 3):
                if g < NG:
                    stage1(g)
            if k + 1 < NPAIR:
                rinv(k + 1)
            for g in (2 * k, 2 * k + 1):
                if g < NG:
                    stage2(g)

        # epilogue: sim = num * (q*n1sq)^-0.5, e = exp(6 sim), sum over frames
        qs_t = epi.tile([NCAP, IF], f32)
        nc.vector.tensor_mul(qs_t, qn_ps[0:NCAP, 0, :], n1sq_t)
        lq_t = epi.tile([NCAP, IF], f32)
        nc.scalar.activation(lq_t, qs_t, FT.Ln, bias=eps_col[0:NCAP, :])
        rq_t = epi.tile([NCAP, IF], f32)
        nc.scalar.activation(rq_t, lq_t, FT.Exp, scale=-0.5)
        sim_t = epi.tile([NCAP, IF], f32)
        nc.vector.tensor_mul(sim_t, qn_ps[0:NCAP, 1, :], rq_t)
        e_t = epi.tile([NCAP, IF], f32)
        nc.scalar.activation(e_t, sim_t, FT.Exp, scale=LAMBDA_LSE)
        se_t = epi.tile([NCAP, IPC], f32)
        nc.vector.reduce_sum(se_t, e_t.rearrange("p (f i) -> p i f", i=IPC),
                             axis=AX.X)
        nc.sync.dma_start(out=se_out, in_=se_t)

    nc.compile()
    return nc


_NC = None


def _get_nc():
    global _NC
    if _NC is None:
        _NC = _build_nc()
    return _NC


def make_in_maps(images, captions, img_lens, cap_lens):
    """Host-side input preparation (numpy only): shard/transpose/mask."""
    images = np.ascontiguousarray(np.asarray(images, np.float32))
    captions = np.ascontiguousarray(np.asarray(captions, np.float32))
    img_lens = np.asarray(img_lens).astype(np.int64)
    cap_lens = np.asarray(cap_lens).astype(np.int64)

    # captions padded to 66; dummies replicate caption 0 (avoids 0/0)
    caps_p = np.concatenate(
        [captions, np.broadcast_to(captions[0:1], (NCAP - N, W, D))], axis=0)
    caps_bf = caps_p.astype(BF16NP)
    # capT [128, NG, DCH, 128]: partition = d % 128 within chunk; word columns
    # padded 120 -> 128 with zeros (weight padding for FWL)
    capT_np = np.zeros((128, NG, DCH, 128), BF16NP)
    capT_np[:, :, :, :GW] = (
        caps_bf.reshape(NG, GP, W, DCH, 128).transpose(4, 0, 3, 1, 2)
        .reshape(128, NG, DCH, GW))

    # per-caption Gram blocks from the bf16-rounded captions (consistency
    # with the on-device bf16 matmuls); block-diagonal per 3-caption group
    cf = caps_bf.astype(np.float32)
    G = np.einsum('jwd,jvd->jwv', cf, cf)                  # (66, 40, 40)
    gpk = np.zeros((128, NG, 128), np.float32)
    for g in range(NG):
        for b in range(GP):
            gpk[b * W:(b + 1) * W, g, b * W:(b + 1) * W] = G[g * GP + b]
    packed_np = np.ascontiguousarray(np.concatenate(
        [capT_np.reshape(128, NG, DCH * 128), gpk.astype(BF16NP)], axis=2))

    wbias_np = np.full((NCAP, W), np.float32(-1e30))
    for j in range(N):
        wbias_np[j, :cap_lens[j]] = 0.0
    wb = np.full((128, NG), np.float32(-1e30))
    wb[:GW] = wbias_np.reshape(NG, GP * W).T
    wbias_np = np.ascontiguousarray(wb)

    # per-group wide indicator: group g's caption b sums into output row 3g+b
    eall_np = np.zeros((128, NG, 128), np.float32)
    for g in range(NG):
        for b in range(GP):
            eall_np[b * W:(b + 1) * W, g, g * GP + b] = 1.0
    eall_np = eall_np.astype(BF16NP)

    in_maps = []
    for core in range(NCORES):
        imgs = images[core * IPC:(core + 1) * IPC].copy()
        for i in range(IPC):
            imgs[i, img_lens[core * IPC + i]:] = 0.0
        imgs_bf = imgs.astype(BF16NP)
        # f-major frame columns: col = f*IPC + i
        Z = np.ascontiguousarray(imgs_bf.transpose(1, 0, 2).reshape(IF, D))
        imgT_np = np.ascontiguousarray(
            Z.reshape(IF, DCH, 128).transpose(2, 1, 0))  # [128, DCH, IF]
        n1 = (Z.astype(np.float32) ** 2).sum(axis=1)     # [IF], f-major
        n1sq_np = np.ascontiguousarray(
            np.broadcast_to(n1[None, :], (NCAP, IF)).astype(np.float32))
        in_maps.append({
            "imgT": imgT_np, "packed": packed_np, "wbias": wbias_np,
            "eall": eall_np, "n1sq": n1sq_np,
        })
    return in_maps


def finish(se_list, img_lens):
    """Host epilogue: defect correction, log-sum-exp, hinge loss."""
    img_lens = np.asarray(img_lens).astype(np.int64)
    cols = []
    for core in range(NCORES):
        se = np.asarray(se_list[core], np.float32)[:N, :]         # (64, 8)
        defect = (F - img_lens[core * IPC:(core + 1) * IPC]).astype(np.float32)
        cols.append(np.log(se - defect[None, :]) / LAMBDA_LSE)
    S = np.concatenate(cols, axis=1).astype(np.float32)           # (caps, imgs)

    diag = np.diag(S)
    eye = np.eye(N, dtype=bool)
    cost_s = np.maximum(MARGIN + S - diag[:, None], 0.0)
    cost_im = np.maximum(MARGIN + S - diag[None, :], 0.0)
    cost_s[eye] = 0.0
    cost_im[eye] = 0.0
    return np.float32(cost_s.max(axis=1).sum() + cost_im.max(axis=0).sum())


def kernel(images, captions, img_lens, cap_lens):
    nc = _get_nc()
    in_maps = make_in_maps(images, captions, img_lens, cap_lens)
    res = run_bass_kernel_spmd(nc, in_maps, core_ids=list(range(NCORES)))
    se_list = [res.results[c]["se_out"] for c in range(NCORES)]
    return finish(se_list, img_lens)
